# revision 41
# baseline (speedup 1.0000x reference)
"""Involution-style per-pixel depthwise 3x3 conv on 8 trn2 NeuronCores.

out[n,c,h,w] = sum_{k=0..8} w[n,c,k,h,w] * x_pad[n,c,h+k//3,w+k%3]  (pad=1)

Sharding: pure data parallel over N=8 -> one sample per core.
Per core: channels C=128 = SBUF partition dim; free dim = H*W pixels.

v5 design — fp16 I/O + host-precomputed DMA-optimal weight layout (the
headroom is memory bandwidth, not precision or flops):

- HBM traffic is the roofline: fp32 I/O is 52 MB/core (~145 us at the
  358 GB/s HBM-per-NC limit). The accuracy budget (rel err < 2e-2)
  doesn't need fp32, so the HOST downcasts x and w to fp16 and upcasts
  the fp16 output, halving device traffic to ~26 MB (~73 us roofline).
  fp16 (not bf16): same DVE/PE throughput, 8x lower rounding error;
  products are bounded (|w*x| < ~50) so no overflow risk. Tap sums
  accumulate exactly in fp32 PSUM; measured rel err ~6e-4.
- Column shift folded into the HOST weight layout: w''_k[h,v] =
  w_k[h,v-dj] (zero-filled border cols). Every DVE product then uses
  the SAME dj=0 x window -> all tensor_mul operands are step-1, 2-byte,
  4B-aligned = the DVE 2x_1P perf-mode trigger (2 elem/cycle/lane).
  The dj shift is repaid as a +-1-element READ OFFSET in the PE
  accumulation stage (PE access patterns have no alignment constraint).
- HOST emits the weights in the exact SBUF image the kernel wants:
  per channel, stripe si's block is [2 zeros][plane_0 n][2 zeros]
  [plane_1 n]...[plane_8 n][2 zeros] (n = rr*W), and the blocks are
  laid out CHUNK-MAJOR in DRAM (each DMA chunk one sequential region).
  Each stripe streams as 3 plain-2D fully-contiguous chunk DMAs of 3
  tap-planes (~9 KB per partition per chunk; the final stripe tapers
  3,3,2,1) — packet count drops ~3x vs strided descriptors and the
  stream runs near the HBM limit, while chunk granularity lets the DVE
  products chase arrivals (an atomic whole-stripe DMA measured a 10 us
  compute drain after the stream; the 1-plane trailing quantum drains
  in ~1 us). The interplane zeros provide the out-of-range targets for
  the PE's +-1 shifted reads (horizontal padding); interior row-wrap
  reads are correct because they land on the host-zeroed border
  columns.
- x lives once in SBUF inside zero guard rows [96 zeros | x | 96 zeros]
  (vertical padding); row overruns of the di=+-1 windows read guards.
- The 9-way tap sum runs on the otherwise-idle TensorE: identity-weight
  fp16 matmuls accumulate all 9 product planes into fp32 PSUM (exact),
  issued TAP-major so the in-order PE queue retires each plane's chunks
  as soon as its product lands (short drain after the final stripe).
  ScalarE evacuates PSUM->SBUF as fp16; stores ride the ACT ring,
  weight loads the SP ring (a store's sem-wait can't head-of-line
  block the weight stream).
- DVE does ONLY the 9 products per stripe (in-place, 2x mode). GPSIMD
  only memsets the x guards: DVE+GPSIMD tensor ops contend on the
  shared SBUF ports and would serialize.
"""

import numpy as np

import concourse.bass as bass
import concourse.mybir as mybir
from concourse.bass_utils import run_bass_kernel_spmd
from concourse.masks import make_identity
from concourse.tile import TileContext

N_CORES = 8
C, H, W = 128, 96, 96
HW = H * W
KW = 3
NK = KW * KW

F32 = mybir.dt.float32
F16 = mybir.dt.float16

# guarded x layout: [one zero row | x | one zero row]
GPAD = W
GX = HW + 2 * GPAD

# small first stripe so the pipeline fills fast; the last stripe's
# DMA chunks taper to a single plane (see _chunks) so the
# products->PE->evac->store drain after the final arrival is short
STRIPE_ROWS = (12, 16, 16, 16, 16, 16, 4)
assert sum(STRIPE_ROWS) == H
SL = max(STRIPE_ROWS) * W  # largest stripe (PSUM/evac tile sizing)


def _block_len(rr):
    """elems per partition of one packed stripe block"""
    return 2 + NK * (rr * W + 2)


def _chunks(rr, last=False):
    """(c0, c1) DMA chunk bounds within a stripe block: three 3-plane
    chunks. Measured optimum — 6-plane or whole-stripe chunks raise
    per-descriptor size but delay the DVE product chase by a larger
    quantum and net +2 to +10 us; single-plane chunks pay too much
    per-packet overhead. The LAST stripe tapers (3,3,2,1): its final
    quantum is one plane, so the post-stream drain is one mul long."""
    p = rr * W + 2
    if last:
        return ((0, 3 * p), (3 * p, 6 * p), (6 * p, 8 * p),
                (8 * p, NK * p + 2))
    return ((0, 3 * p), (3 * p, 6 * p), (6 * p, NK * p + 2))


W_LEN = sum(_block_len(rr) for rr in STRIPE_ROWS)


def _build() -> bass.Bass:
    nc = bass.Bass()
    x_d = nc.dram_tensor("x", [C, HW], F16, kind="ExternalInput")
    w_d = nc.dram_tensor("w", [C, W_LEN], F16, kind="ExternalInput")
    o_d = nc.dram_tensor("out", [C, HW], F16, kind="ExternalOutput")

    with TileContext(nc) as tc:
        with (
            tc.tile_pool(name="px", bufs=1) as px,
            tc.tile_pool(name="pw", bufs=6) as pw,
            tc.tile_pool(name="pg", bufs=2) as pg,
            tc.tile_pool(name="pp", bufs=2, space="PSUM") as pp,
        ):
            # x first on the ACT ring (head covers stripe 0's di=+1
            # window, rows <= 9); the weight stream owns the SP ring —
            # the SP HWDGE measures ~1.7x the ACT ring's rate on this
            # pattern, so everything else stays off it.
            xg = px.tile([C, GX], F16)
            Q = 10 * W
            nc.scalar.dma_start(out=xg[:, GPAD : GPAD + Q], in_=x_d[:, 0:Q])
            nc.gpsimd.memset(xg[:, 0:GPAD], 0.0)
            nc.gpsimd.memset(xg[:, GPAD + HW : GX], 0.0)

            # Each stripe streams as 3 chunk-DMAs of 3 tap-planes each
            # (still plain 2D contiguous, ~9 KB per partition): the DVE
            # products chase chunk arrivals instead of waiting for the
            # whole ~3.5 MB stripe, so compute trails the stream by one
            # chunk and the post-stream drain stays ~2 planes long.
            # The host emits the chunks CHUNK-MAJOR in DRAM: all 128
            # partitions' data for a chunk is one sequential ~1.2 MB
            # region (partition stride = chunk length), so the SDMA
            # engines read fully sequential HBM addresses.
            wt = w_d[:, 0:1]
            slab_tiles = []
            flat = 0
            for si, rr in enumerate(STRIPE_ROWS):
                blen = _block_len(rr)
                pitch = rr * W + 2
                slab = pw.tile([C, blen], F16, tag="w", name=f"w_{si}")
                last = si == len(STRIPE_ROWS) - 1
                for c0, c1 in _chunks(rr, last):
                    L = c1 - c0
                    src = bass.AP(wt.tensor, flat, [[L, C], [1, L]])
                    nc.sync.dma_start(out=slab[:, c0:c1], in_=src)
                    flat += C * L
                slab_tiles.append(slab)
                if si == 0:
                    nc.scalar.dma_start(
                        out=xg[:, GPAD + Q : GPAD + HW], in_=x_d[:, Q:HW]
                    )

            ident_f = px.tile([C, C], F32)
            make_identity(nc, ident_f)
            ident = px.tile([C, C], F16)
            nc.vector.tensor_copy(out=ident[:, :], in_=ident_f[:, :])

            r0 = 0
            for si, rr in enumerate(STRIPE_ROWS):
                n = rr * W
                pitch = n + 2
                slab = slab_tiles[si]

                # in-place products; tap k's plane sits at 2 + k*pitch.
                # Every tap of row-group di multiplies the SAME x window
                # (the dj shift lives in the host w layout), so all
                # operands are step-1 / 2B / 4B-aligned -> DVE 2x mode.
                # Tap order follows the chunk stream (0..8) so each mul's
                # wait is satisfied as its chunk lands.
                taps = (0, 1, 2, 3, 4, 5, 6, 7, 8)
                for k in taps:
                    di = k // KW - 1
                    b = 2 + k * pitch
                    s = GPAD + (r0 + di) * W
                    nc.vector.tensor_mul(
                        out=slab[:, b : b + n],
                        in0=slab[:, b : b + n],
                        in1=xg[:, s : s + n],
                    )

                # 9-tap sum on TensorE: identity matmuls accumulate the
                # product planes into fp32 PSUM; plane k is read at the
                # +-1 offset dj (the host-shift repayment), landing on
                # the interplane zeros at the edges. TAP-major issue so
                # the PE chases the DVE plane by plane.
                acc_ps = pp.tile([C, SL], F32, tag="acc", space="PSUM")
                n_ft = (n + 511) // 512
                for i_t, k in enumerate(taps):
                    dj = k % KW - 1
                    b = 2 + k * pitch + dj
                    for j in range(n_ft):
                        f0, f1 = j * 512, min((j + 1) * 512, n)
                        nc.tensor.matmul(
                            acc_ps[:, f0:f1],
                            ident[:, :],
                            slab[:, b + f0 : b + f1],
                            start=(i_t == 0),
                            stop=(i_t == NK - 1),
                        )

                # evacuate PSUM -> SBUF as fp16 on ScalarE (own ports),
                # store on the ACT ring
                stg = pg.tile([C, SL], F16, tag="stg")
                nc.scalar.copy(out=stg[:, 0:n], in_=acc_ps[:, 0:n])
                nc.scalar.dma_start(
                    out=o_d[:, r0 * W : (r0 + rr) * W], in_=stg[:, 0:n]
                )
                r0 += rr

    return nc


def _split_excess_waits(nc: bass.Bass) -> None:
    """TPB engine instructions carry exactly ONE sync-wait slot; walrus
    refuses instructions with more ("Too many sync wait commands"). Tile's
    sem assignment can emit several waits on one instruction. Split the
    extras onto same-engine NOPs inserted immediately before the
    instruction — the engine sequencer executes them in order, so all
    waits are still satisfied before the instruction runs."""
    import bass_rust

    f = nc.m.functions[0]

    def make_nop(engine):
        ins = nc.engines[engine].nop().ins
        # nop() appends to the currently-open bb; detach it from there
        for bb in f.blocks:
            il = bb.instructions
            for j in range(len(il) - 1, -1, -1):
                if il[j].name == ins.name:
                    del il[j]
                    return ins
        raise AssertionError("freshly created nop not found in any block")

    for bb in f.blocks:
        il = bb.instructions
        i = 0
        while i < len(il):
            ins = il[i]
            si = ins.sync_info
            waits = list(si.on_wait) if si and si.on_wait else []
            if len(waits) > 1:
                updates = list(si.on_update) if si.on_update else []
                ins.sync_info = bass_rust.SyncInfo(
                    on_wait=[waits[-1]], on_update=updates
                )
                for k, w in enumerate(waits[:-1]):
                    nop = make_nop(ins.engine)
                    nop.sync_info = bass_rust.SyncInfo(on_wait=[w], on_update=[])
                    il.insert(i + k, nop)
                i += len(waits) - 1
            i += 1


_NC_CACHE = None


def _get_nc():
    global _NC_CACHE
    if _NC_CACHE is None:
        nc = _build()
        _split_excess_waits(nc)
        _NC_CACHE = nc
    return _NC_CACHE


_RUNNER = None


def _get_runner():
    """Jit the SPMD executable once; repeated kernel() calls reuse it.

    Mirrors concourse.bass2jax.run_bass_via_pjrt's multi-core branch but
    caches the jitted callable (run_bass_via_pjrt builds a fresh closure
    per call, forcing an XLA recompile every time)."""
    global _RUNNER
    if _RUNNER is not None:
        return _RUNNER

    import jax
    from jax.experimental.shard_map import shard_map
    from jax.sharding import Mesh, PartitionSpec

    import concourse.mybir as _mybir
    from concourse import bass2jax

    bass2jax.install_neuronx_cc_hook()
    nc = _get_nc()

    partition_name = (
        nc.partition_id_tensor.name if nc.partition_id_tensor else None
    )
    in_names, out_names, out_avals = [], [], []
    for alloc in nc.m.functions[0].allocations:
        if not isinstance(alloc, _mybir.MemoryLocationSet):
            continue
        name = alloc.memorylocations[0].name
        if alloc.kind == "ExternalInput":
            if name != partition_name:
                in_names.append(name)
        elif alloc.kind == "ExternalOutput":
            out_names.append(name)
            out_avals.append(
                jax.core.ShapedArray(
                    tuple(alloc.tensor_shape), _mybir.dt.np(alloc.dtype)
                )
            )
    n_params = len(in_names)
    n_outs = len(out_names)
    all_in_names = tuple(in_names + out_names)
    if partition_name is not None:
        all_in_names = all_in_names + (partition_name,)
    donate = tuple(range(n_params, n_params + n_outs))

    def _body(*args):
        operands = list(args)
        if partition_name is not None:
            operands.append(bass2jax.partition_id_tensor())
        outs = bass2jax._bass_exec_p.bind(
            *operands,
            out_avals=tuple(out_avals),
            in_names=all_in_names,
            out_names=tuple(out_names),
            lowering_input_output_aliases=(),
            sim_require_finite=True,
            sim_require_nnan=True,
            nc=nc,
        )
        return tuple(outs)

    devices = jax.devices()[:N_CORES]
    mesh = Mesh(np.asarray(devices), ("core",))
    sharded = jax.jit(
        shard_map(
            _body,
            mesh=mesh,
            in_specs=(PartitionSpec("core"),) * (n_params + n_outs),
            out_specs=(PartitionSpec("core"),) * n_outs,
            check_rep=False,
        ),
        donate_argnums=donate,
        keep_unused=True,
    )

    def runner(concat_inputs):
        zeros = [
            np.zeros((N_CORES * a.shape[0], *a.shape[1:]), a.dtype) for a in out_avals
        ]
        outs = sharded(*concat_inputs, *zeros)
        return [np.asarray(o) for o in outs]

    _RUNNER = (runner, in_names, out_names, out_avals)
    return _RUNNER


def _host_prep(x, conv_weights):
    """fp32 -> fp16 downcast + the packed, column-shifted weight image.

    w''_k[h,v] = w_k[h,v-dj] with zero-filled border columns (so the
    device multiplies every tap against the unshifted x window and the
    PE accumulation reads plane k at offset dj), then per (channel,
    stripe) the 9 tap planes are laid out contiguously with 2-element
    zero pads: [00][plane_0][00][plane_1]...[plane_8][00]. The pads are
    the in-bounds zero targets for the PE's +-1 shifted edge reads.
    """
    x = np.asarray(x)
    w = np.asarray(conv_weights)
    assert x.shape == (N_CORES, C, H, W), x.shape
    assert w.shape == (N_CORES, C * NK, H, W), w.shape
    x16 = x.astype(np.float16)
    wr = w.reshape(N_CORES, C, NK, H, W)
    w16 = np.zeros((N_CORES, C, NK, H, W), dtype=np.float16)
    w16[:, :, 1::3] = wr[:, :, 1::3]                      # dj=0
    w16[:, :, 0::3, :, : W - 1] = wr[:, :, 0::3, :, 1:]   # dj=-1: shift left
    w16[:, :, 2::3, :, 1:] = wr[:, :, 2::3, :, : W - 1]   # dj=+1: shift right

    w_packed = np.zeros((N_CORES, C, W_LEN), dtype=np.float16)
    b = 0
    r0 = 0
    for rr in STRIPE_ROWS:
        n = rr * W
        blen = _block_len(rr)
        # stripe block: 2 leading zeros + 9 planes at pitch n+2
        blk = w_packed[:, :, b + 2 : b + blen].reshape(N_CORES, C, NK, n + 2)
        blk[:, :, :, :n] = w16[:, :, :, r0 : r0 + rr].reshape(
            N_CORES, C, NK, n
        )
        b += blen
        r0 += rr
    # reorder CHUNK-MAJOR to match the device's sequential-DRAM APs:
    # per core the flat stream is [stripe 0 chunk 0: all 128 channels]
    # [stripe 0 chunk 1: ...] ... — each chunk one sequential region
    segs = []
    b = 0
    for si, rr in enumerate(STRIPE_ROWS):
        blen = _block_len(rr)
        for c0, c1 in _chunks(rr, si == len(STRIPE_ROWS) - 1):
            seg = w_packed[:, :, b + c0 : b + c1]
            segs.append(seg.reshape(N_CORES, C * (c1 - c0)))
        b += blen
    w_flat = np.concatenate(segs, axis=1)
    return {
        "x": np.ascontiguousarray(x16.reshape(N_CORES * C, HW)),
        "w": np.ascontiguousarray(w_flat.reshape(N_CORES * C, W_LEN)),
    }


def prep_inputs(x, conv_weights):
    """Reshape full inputs into the concatenated per-core layout."""
    by_name = _host_prep(x, conv_weights)
    _, in_names, _, _ = _get_runner()
    return [by_name[n] for n in in_names]


def execute(concat_inputs):
    runner, _, out_names, out_avals = _get_runner()
    outs = runner(concat_inputs)
    i = out_names.index("out")
    return outs[i].reshape(N_CORES, C, H, W).astype(np.float32)


def kernel(x, conv_weights):
    return execute(prep_inputs(x, conv_weights))


def run(x, conv_weights, **spmd_kwargs):
    """Legacy full-path entry via run_bass_kernel_spmd (no jit caching)."""
    by_name = _host_prep(x, conv_weights)
    xs, ws = by_name["x"], by_name["w"]
    n = N_CORES
    nc = _get_nc()
    in_maps = [
        {
            "x": xs[i * C : (i + 1) * C],
            "w": ws[i * C : (i + 1) * C],
        }
        for i in range(n)
    ]
    br = run_bass_kernel_spmd(nc, in_maps, core_ids=list(range(n)), **spmd_kwargs)
    out = np.stack(
        [r["out"].reshape(C, H, W).astype(np.float32) for r in br.results]
    )
    return out, br
